# revision 37
# baseline (speedup 1.0000x reference)
"""Trainium2 Bass kernel for nn_MultiHeadAttention (SL=2048, BS=2, D=1024, H=16, DH=64).

Sharding: the [BS=2, H=16] grid of attention heads is split across 8 cores:
core c handles batch b = c//4 and heads 4*(c%4) .. 4*(c%4)+4.
Each core computes q/k/v projections for its own head slice, the 4 attention
maps, and a partial output (its heads' contribution through Wo). The host
sums the 4 partials per batch.

Pipeline structure (per core): 8 softmax "ladders" ordered hp-major
(L = hp*4 + qc over head-pairs hp and 512-query chunks qc). The pre-phase
computes only the k-projection for head-pair 0 (ot=0) plus the first query
chunk, so ladder 0 starts ~10us in; all remaining q/k projections and the
Wo output projections are spread as PE inserts across later ladders where
the exp-activation (the ACT-bound inner loop) leaves tensor-engine slack.

Scores are computed transposed (S^T[k, q]) in f16 so softmax-exp output
feeds the AV matmul directly; softmax denominators ride along as a 65th
"ones" row of the V stationary, are broadcast to 64 partitions with a
0-stride DMA, inverted on DVE, and applied as an elementwise multiply.
"""

import os
import ml_dtypes
import numpy as np

SL, BS, D = 2048, 2, 1024
H, DH = 16, 64
NCORES = 8
HPC = 4            # heads per core
OD = HPC * DH      # 256 projected dims per core
DC = D // 128      # 8 contraction chunks
QC = SL // 512     # 4 query chunks of 512
KT = SL // 128     # 16 key tiles of 128

_NC = None
LAST_RESULT = None


def _build_nc():
    import concourse.mybir as mybir
    import concourse.tile as tile
    from concourse import bacc

    f32 = mybir.dt.float32
    f32r = mybir.dt.float32r
    bf16 = mybir.dt.bfloat16
    f16 = mybir.dt.float16
    EXP = mybir.ActivationFunctionType.Exp

    nc = bacc.Bacc(None, target_bir_lowering=False, debug=True)

    # x inputs are pre-chunked on the host so every SBUF tile is one
    # contiguous DRAM block: xq as [qc, d, 128, 512], xk/xv as
    # [cc2, d, 128, 1024], flattened to 2D.
    xqT = nc.dram_tensor("xqT", [QC * DC * 128, 512], f16, kind="ExternalInput")
    xkT = nc.dram_tensor("xkT", [2 * DC * 128, 1024], f16, kind="ExternalInput")
    xvT = nc.dram_tensor("xvT", [2 * DC * 128, 1024], f16, kind="ExternalInput")
    wqT = nc.dram_tensor("wqT", [128, DC * OD], f16, kind="ExternalInput")
    wkT = nc.dram_tensor("wkT", [128, DC * OD], f16, kind="ExternalInput")
    wvT = nc.dram_tensor("wvT", [128, DC * OD], f16, kind="ExternalInput")
    woT = nc.dram_tensor("woT", [128, 2 * D], f16, kind="ExternalInput")
    onesd = nc.dram_tensor("onesd", [128, 260], f32r, kind="ExternalInput")
    onesvd = nc.dram_tensor("onesvd", [128, 260], bf16, kind="ExternalInput")
    yT = nc.dram_tensor("yT", [D, SL], f32, kind="ExternalOutput")

    with tile.TileContext(nc) as tc:
        with (
            tc.tile_pool(name="wsb", bufs=1) as wsb,
            tc.tile_pool(name="qk", bufs=1) as qk,
            tc.tile_pool(name="vsb", bufs=1) as vsb,
            tc.tile_pool(name="xkp", bufs=16) as xkp,
            tc.tile_pool(name="xqp", bufs=32) as xqp,
            tc.tile_pool(name="xvp", bufs=16) as xvp,
            tc.tile_pool(name="esb", bufs=6) as esb,
            tc.tile_pool(name="otsb", bufs=8) as otsb,
            tc.tile_pool(name="ysb", bufs=4) as ysb,
            tc.tile_pool(name="yap", bufs=1) as yap,
            tc.tile_pool(name="otmp", bufs=2) as otmp,
            tc.tile_pool(name="avsb", bufs=2) as avsb,
            tc.tile_pool(name="pp", bufs=1, space="PSUM") as pp,
            tc.tile_pool(name="wp", bufs=2, space="PSUM") as wp,
            tc.tile_pool(name="avop", bufs=1, space="PSUM") as avop,
            tc.tile_pool(name="yp", bufs=1, space="PSUM") as yp,
        ):
            # --- persistent SBUF tensors ---
            wq_sb = wsb.tile([128, DC * OD], f16, tag="wq")  # [p, dc*256+od]
            wk_sb = wsb.tile([128, DC * OD], f16, tag="wk")
            wv_sb = wsb.tile([128, DC * OD], f16, tag="wv")
            wo_sb = wsb.tile([128, 2 * D], f16, tag="wo")    # [p, hp*1024+o]
            ones_sb = wsb.tile([128, 260], f32r, tag="ones")
            kT_sb = [qk.tile([128, SL], f16, tag=f"kT{ot}", name=f"kT{ot}") for ot in range(2)]
            qT_sb = [qk.tile([128, SL], f16, tag=f"qT{ot}", name=f"qT{ot}") for ot in range(2)]
            v_sb = [vsb.tile([128, 260], bf16, tag=f"v{t}", name=f"v{t}") for t in range(KT)]

            def load_w(dst, src):
                nc.sync.dma_start(out=dst[:], in_=src[:])

            def load_x(pool, tg, xdram, cc, dt_=f16):
                tiles = []
                for d in range(DC):
                    r = (cc * DC + d) * 128
                    t = pool.tile([128, 512], dt_, tag=tg, name="x")
                    nc.sync.dma_start(out=t[:], in_=xdram[r:r + 128, 0:512])
                    tiles.append(t[:])
                return tiles

            def load_x2(pool, tg, xdram, cc2, dt_=f16):
                # each [128, 1024] tile is one contiguous DRAM block
                ev, od = [], []
                for d in range(DC):
                    r = (cc2 * DC + d) * 128
                    t = pool.tile([128, 1024], dt_, tag=tg, name="x")
                    nc.sync.dma_start(out=t[:], in_=xdram[r:r + 128, 0:1024])
                    ev.append(t[:, 0:512])
                    od.append(t[:, 512:1024])
                return ev, od

            qp_ps = {}

            def proj_qk_quarter(w_sb, dst, cc, xtiles, ot, half):
                if half == 0:
                    qp_ps[(cc, ot)] = pp.tile([128, 512], f32, tag="pp",
                                              name="ps")
                ps = qp_ps[(cc, ot)]
                for d in range(half * 4, half * 4 + 4):
                    nc.tensor.matmul(
                        ps[:],
                        (w_sb[:, d * OD + ot * 128: d * OD + (ot + 1) * 128]),
                        (xtiles[d][:]),
                        start=(d == 0), stop=(d == DC - 1))
                if half == 1:
                    nc.vector.tensor_copy(dst[ot][:, cc * 512:(cc + 1) * 512],
                                          ps[:])

            def proj_tile(w_sb, dst, cc, xtiles, ot):
                proj_qk_quarter(w_sb, dst, cc, xtiles, ot, 0)
                proj_qk_quarter(w_sb, dst, cc, xtiles, ot, 1)

            def emit_wo_piece(qc_, ot_tiles, pool, ptag, o8):
                Y = pool.tile([128, 512], f32, tag=ptag, name="Y")
                for hp in range(2):
                    nc.tensor.matmul(
                        Y[:],
                        (wo_sb[:, hp * D + o8 * 128: hp * D + (o8 + 1) * 128]),
                        (ot_tiles[hp][:]),
                        start=(hp == 0), stop=(hp == 1))
                ys = ysb.tile([128, 512], f32, tag="ys", name="ys")
                nc.vector.tensor_copy(ys[:], Y[:])
                nc.sync.dma_start(
                    out=yT[o8 * 128:(o8 + 1) * 128, qc_ * 512:(qc_ + 1) * 512],
                    in_=ys[:])

            def emit_wo(qc_, ot_tiles, pools):
                for o8 in range(8):
                    pool, ptag = pools[o8 % len(pools)]
                    emit_wo_piece(qc_, ot_tiles, pool, ptag, o8)

            # --- pre-phase: warmup, k-proj ot=0, q-proj qc0 ot=0 ---
            nc.sync.dma_start(out=ones_sb[:], in_=onesd[:])
            load_w(wk_sb, wkT)
            warm = yp.tile([128, 512], f32, tag="yp", name="warm")
            for i in range(12):
                nc.tensor.matmul(warm[0:64, 0:256], ones_sb[:, 0:64],
                                 ones_sb[:, 0:256], start=(i == 0),
                                 stop=(i == 11))
            warms = ysb.tile([64, 256], f32, tag="ys", name="warms")
            nc.vector.tensor_copy(warms[:], warm[0:64, 0:256])

            ev0, od0 = load_x2(xkp, 'xk', xkT, 0, f16)
            load_w(wq_sb, wqT)
            xq_t = {0: load_x(xqp, 'xq', xqT, 0, f16)}
            load_w(wv_sb, wvT)
            ev1, od1 = load_x2(xkp, 'xk', xkT, 1, f16)
            xk_cc = [ev0, od0, ev1, od1]

            proj_tile(wk_sb, kT_sb, 0, xk_cc[0], 0)
            proj_tile(wk_sb, kT_sb, 1, xk_cc[1], 0)
            proj_tile(wq_sb, qT_sb, 0, xq_t[0], 0)

            nc.sync.dma_start(out=wo_sb[:], in_=woT[:])
            _xv01 = load_x2(xvp, 'xv', xvT, 0, f16)
            xv_chunks = {0: _xv01[0], 1: _xv01[1]}

            def emit_vtile(t_):
                cc_, tt = divmod(t_, 4)
                xtiles = xv_chunks[cc_]
                nc.sync.dma_start(out=v_sb[t_][:], in_=onesvd[:])
                ps = yp.tile([128, OD], f32, tag="yp", name="vps")
                for d in range(DC):
                    nc.tensor.matmul(
                        ps[:],
                        (xtiles[d][:, tt * 128:(tt + 1) * 128]),
                        (wv_sb[:, d * OD:(d + 1) * OD]),
                        start=(d == 0), stop=(d == DC - 1))
                for h in range(4):
                    nc.vector.tensor_copy(
                        v_sb[t_][:, h * 65:h * 65 + 64],
                        ps[:, h * 64:(h + 1) * 64])

            # --- insert schedule -------------------------------------------
            inserts = {}

            def at(L_, kt_, fn):
                inserts.setdefault((L_, kt_), []).append(fn)

            def load_xq(qc_):
                xq_t[qc_] = load_x(xqp, 'xq', xqT, qc_, f16)

            # remaining k-proj chunks for ot=0 (needed by ladder-0 kt>=8)
            at(0, 0, (lambda: proj_tile(wk_sb, kT_sb, 2, xk_cc[2], 0)))
            at(0, 1, (lambda: load_xq(1)))
            at(0, 2, (lambda: proj_tile(wk_sb, kT_sb, 3, xk_cc[3], 0)))
            at(0, 6, (lambda: proj_tile(wq_sb, qT_sb, 1, xq_t[1], 0)))
            # k-proj ot=1 (needed by ladder 4) and remaining q-projections
            at(1, 0, (lambda: proj_tile(wk_sb, kT_sb, 0, xk_cc[0], 1)))
            at(1, 1, (lambda: load_xq(2)))
            at(1, 5, (lambda: proj_tile(wq_sb, qT_sb, 2, xq_t[2], 0)))
            at(2, 0, (lambda: proj_tile(wk_sb, kT_sb, 1, xk_cc[1], 1)))
            at(2, 1, (lambda: load_xq(3)))
            at(2, 4, (lambda: proj_tile(wq_sb, qT_sb, 3, xq_t[3], 0)))
            at(3, 0, (lambda: proj_tile(wk_sb, kT_sb, 2, xk_cc[2], 1)))
            at(3, 8, (lambda: proj_tile(wq_sb, qT_sb, 0, xq_t[0], 1)))
            at(4, 0, (lambda: proj_tile(wk_sb, kT_sb, 3, xk_cc[3], 1)))
            at(4, 8, (lambda: proj_tile(wq_sb, qT_sb, 1, xq_t[1], 1)))
            at(5, 0, (lambda: proj_tile(wq_sb, qT_sb, 2, xq_t[2], 1)))
            at(6, 0, (lambda: proj_tile(wq_sb, qT_sb, 3, xq_t[3], 1)))

            # Wo(qc3) hp0-halves are staged to SBUF during ladder 7 so the
            # tail after the last exp only runs the hp1 matmuls + adds.
            ya_sb = {}

            def emit_wo_qc3_hp0(o8):
                pool, ptag = ((yp, 'yp'), (pp, 'pp'))[o8 % 2]
                Y = pool.tile([128, 512], f32, tag=ptag, name="Ya")
                nc.tensor.matmul(
                    Y[:],
                    (wo_sb[:, o8 * 128:(o8 + 1) * 128]),
                    (OTs[(QC - 1, 0)][:]),
                    start=True, stop=True)
                ya = yap.tile([128, 512], f32, tag=f"ya{o8}", name="ya")
                nc.vector.tensor_copy(ya[:], Y[:])
                ya_sb[o8] = ya

            for _o in range(8):
                at(7, 8 + _o, (lambda o=_o: emit_wo_qc3_hp0(o)))

            OTs = {}

            def emit_scores(L_, kt_):
                hp_, qc_ = divmod(L_, QC)
                W = wp.tile([128, 1024], f32, tag="wp", name="W")
                for hip in range(2):
                    nc.tensor.matmul(
                        W[:, hip * 512:(hip + 1) * 512],
                        (kT_sb[hp_][hip * 64:(hip + 1) * 64,
                                     kt_ * 128:(kt_ + 1) * 128]),
                        (qT_sb[hp_][hip * 64:(hip + 1) * 64,
                                     qc_ * 512:(qc_ + 1) * 512]),
                        start=True, stop=True)
                return W

            Ws = {}

            # --- attention: 8 ladders, hp-major (L = hp*4 + qc) ---
            for L in range(2 * QC):
                hp, qc = divmod(L, QC)
                AVO = [avop.tile([65, 512], f32, tag=f"av{hip}", name="AVO")
                       for hip in range(2)]

                def emit_av(E_, kt_, AVO=AVO, hp=hp):
                    for hip in range(2):
                        nc.tensor.matmul(
                            AVO[hip][:],
                            (v_sb[kt_][:, (hp * 2 + hip) * 65:
                                         (hp * 2 + hip) * 65 + 65]),
                            (E_[:, hip * 512:(hip + 1) * 512]),
                            start=(kt_ == 0), stop=(kt_ == KT - 1))

                prev_E = None
                for kt in range(KT):
                    if (L, kt) == (0, 0):
                        Ws[(0, 0)] = emit_scores(0, 0)
                    W = Ws.pop((L, kt))
                    E = esb.tile([128, 1024], bf16, tag="E", name="E")
                    nc.scalar.activation(E[:], W[:], EXP)
                    # emit the NEXT scores pair right after the exp so it
                    # outranks AV/v-proj/insert matmuls in scheduler priority
                    # and the next exp is never starved behind them
                    if kt < KT - 1:
                        Ws[(L, kt + 1)] = emit_scores(L, kt + 1)
                    elif L < 2 * QC - 1:
                        Ws[(L + 1, 0)] = emit_scores(L + 1, 0)
                    if prev_E is not None:
                        emit_av(prev_E, kt - 1)
                    prev_E = E
                    if L == 0:
                        emit_vtile(kt)
                        if kt == 1:
                            _xv23 = load_x2(xvp, 'xv', xvT, 1, f16)
                            xv_chunks[2] = _xv23[0]
                            xv_chunks[3] = _xv23[1]
                    for fn in inserts.pop((L, kt), []):
                        fn()
                emit_av(prev_E, KT - 1)

                # evacuate accumulators promptly, then defer the normalize
                # chain into the next ladder
                avs_pair = []
                for hip in range(2):
                    avs = avsb.tile([65, 512], f32, tag="avs", name="avs")
                    nc.vector.tensor_copy(avs[:], AVO[hip][:])
                    avs_pair.append(avs)
                OT = otsb.tile([128, 512], f16, tag="ot", name="OT")
                OTs[(qc, hp)] = OT

                def chain(hip, avs_pair=avs_pair, OT=OT):
                    avs = avs_pair[hip]
                    srow = ysb.tile([1, 512], f32, tag="ys", name="srow")
                    nc.sync.dma_start(out=srow[:], in_=avs[64:65, :])
                    sumsb = ysb.tile([64, 512], f32, tag="ys", name="sumsb")
                    nc.gpsimd.partition_broadcast(sumsb[:], srow[:])
                    BCs = ysb.tile([64, 512], f32, tag="ys", name="BCs")
                    nc.vector.reciprocal_approx_fast(BCs[:], sumsb[:])
                    if hip == 0:
                        nc.vector.tensor_mul(OT[0:64, :], avs[0:64, :], BCs[:])
                    else:
                        OTt = otmp.tile([64, 512], f16, tag="otmp", name="OTt")
                        nc.vector.tensor_mul(OTt[:], avs[0:64, :], BCs[:])
                        nc.sync.dma_start(out=OT[64:128, :], in_=OTt[:])

                if L < 2 * QC - 1:
                    at(L + 1, 1, (lambda c=chain: c(0)))
                    at(L + 1, 3, (lambda c=chain: c(1)))
                else:
                    # hip1's chain has the extra OT-assembly DMA hop; start
                    # it first so the tail Wo isn't waiting on it
                    chain(1)
                    chain(0)

                # Wo(qc) can start once OT(qc, hp=1) is written: its chain
                # runs at ladder L'=4+qc+1 slots 1/3, so spread the pieces
                # over slots 5..12 of that ladder, alternating PSUM pools.
                if hp == 1 and qc < QC - 1:
                    for o8 in range(8):
                        pool, ptag = ((yp, 'yp'), (pp, 'pp'))[o8 % 2]
                        at(L + 1, 5 + o8,
                           (lambda q=qc, o=o8, p=pool, pt=ptag:
                            emit_wo_piece(q, [OTs[(q, 0)], OTs[(q, 1)]],
                                          p, pt, o)))

            # tail: hp1 halves of Wo(qc3), added to the staged hp0 partials
            tail_pools = [(yp, 'yp'), (pp, 'pp'), (avop, 'av0'), (avop, 'av1')]
            for o8 in range(8):
                pool, ptag = tail_pools[o8 % 4]
                Y = pool.tile([128, 512], f32, tag=ptag, name="Yb")
                nc.tensor.matmul(
                    Y[:],
                    (wo_sb[:, D + o8 * 128:D + (o8 + 1) * 128]),
                    (OTs[(QC - 1, 1)][:]),
                    start=True, stop=True)
                ys = ysb.tile([128, 512], f32, tag="ys", name="ys")
                nc.vector.tensor_add(ys[:], ya_sb[o8][:], Y[:])
                nc.sync.dma_start(
                    out=yT[o8 * 128:(o8 + 1) * 128,
                           (QC - 1) * 512:QC * 512],
                    in_=ys[:])

    nc.compile()
    return nc


def _get_nc():
    global _NC
    if _NC is None:
        _NC = _build_nc()
    return _NC


def _host_fallback(query, keys, values, mask, Wq, Wk, Wv, Wo):
    # Exact reference math in numpy; only used if mask has zeros (off-spec).
    q = (query @ Wq.T).reshape(SL, BS, H, DH)
    k = (keys @ Wk.T).reshape(SL, BS, H, DH)
    v = (values @ Wv.T).reshape(SL, BS, H, DH)
    out = np.zeros((SL, BS, H * DH), np.float32)
    for b in range(BS):
        for h in range(H):
            s = q[:, b, h, :] @ k[:, b, h, :].T
            s = np.where(mask[0, 0] == 0, np.float32(-1e20), s)
            s = s - s.max(axis=-1, keepdims=True)
            p = np.exp(s)
            p /= p.sum(axis=-1, keepdims=True)
            out[:, b, h * DH:(h + 1) * DH] = p @ v[:, b, h, :]
    return out @ Wo.T


def _enable_trace_support():
    """Install the antenv.axon_hooks shim so trace=True works under axon."""
    import sys
    import types
    import antenv
    if "antenv.axon_hooks" in sys.modules:
        return
    hookmod = types.ModuleType("antenv.axon_hooks")
    _hook = [None]
    hookmod.set_axon_ntff_profile_hook = lambda h: _hook.__setitem__(0, h)
    hookmod.get_axon_ntff_profile_hook = lambda: _hook[0]
    antenv.axon_hooks = hookmod
    sys.modules["antenv.axon_hooks"] = hookmod
    try:
        from trn_agent_boot.trn_boot import _ntff_profile_via_ctypes
        hookmod.set_axon_ntff_profile_hook(
            _ntff_profile_via_ctypes("/opt/axon/libaxon_pjrt.so"))
    except Exception:
        pass
    import concourse.bass_utils as bu
    bu.upload_artifacts = lambda tmpdir: tmpdir


def _w_sb_layout(Wslice):
    # [256 od, 1024 D] -> [128 p, dc*256+od]
    return np.ascontiguousarray(
        Wslice.reshape(OD, DC, 128).transpose(2, 1, 0).reshape(128, DC * OD))


def _wo_sb_layout(WoSlice):
    # [1024 o, 256 hd] -> [128 p, hp*1024+o]
    return np.ascontiguousarray(
        WoSlice.reshape(D, 2, 128).transpose(2, 1, 0).reshape(128, 2 * D))


def kernel(query, keys, values, mask, Wq, Wk, Wv, Wo):
    query = np.asarray(query, np.float32)
    keys = np.asarray(keys, np.float32)
    values = np.asarray(values, np.float32)
    mask = np.asarray(mask)
    Wq = np.asarray(Wq, np.float32)
    Wk = np.asarray(Wk, np.float32)
    Wv = np.asarray(Wv, np.float32)
    Wo = np.asarray(Wo, np.float32)

    if (mask == 0).any():
        return _host_fallback(query, keys, values, mask, Wq, Wk, Wv, Wo)

    trace = bool(int(os.environ.get("KERNEL_TRACE", "0")))
    if trace:
        _enable_trace_support()

    from concourse.bass_utils import run_bass_kernel_spmd

    nc = _get_nc()
    in_maps = []
    for c in range(NCORES):
        b, hg = divmod(c, 4)
        hs = hg * OD
        xq_c = query[:, b, :].T.reshape(DC, 128, QC, 512).transpose(
            2, 0, 1, 3).reshape(QC * DC * 128, 512)
        xk_c = keys[:, b, :].T.reshape(DC, 128, 2, 1024).transpose(
            2, 0, 1, 3).reshape(2 * DC * 128, 1024)
        xv_c = values[:, b, :].T.reshape(DC, 128, 2, 1024).transpose(
            2, 0, 1, 3).reshape(2 * DC * 128, 1024)
        in_maps.append({
            "xqT": np.ascontiguousarray(xq_c).astype(np.float16),
            "xkT": np.ascontiguousarray(xk_c).astype(np.float16),
            "xvT": np.ascontiguousarray(xv_c).astype(np.float16),
            "wqT": _w_sb_layout(Wq[hs:hs + OD, :]).astype(np.float16),
            "wkT": _w_sb_layout(Wk[hs:hs + OD, :]).astype(np.float16),
            "wvT": _w_sb_layout(Wv[hs:hs + OD, :]).astype(np.float16),
            "woT": _wo_sb_layout(Wo[:, hs:hs + OD]).astype(np.float16),
            "onesd": np.ones((128, 260), np.float32),
            "onesvd": np.ones((128, 260), ml_dtypes.bfloat16),
        })

    res = run_bass_kernel_spmd(nc, in_maps, core_ids=list(range(NCORES)),
                               trace=trace)
    global LAST_RESULT
    LAST_RESULT = res

    out = np.zeros((SL, BS, D), np.float32)
    for c in range(NCORES):
        b = c // 4
        out[:, b, :] += res.results[c]["yT"].T
    return out


# revision 38
# speedup vs baseline: 1.0163x; 1.0163x over previous
"""Trainium2 Bass kernel for nn_MultiHeadAttention (SL=2048, BS=2, D=1024, H=16, DH=64).

Sharding: the [BS=2, H=16] grid of attention heads is split across 8 cores:
core c handles batch b = c//4 and heads 4*(c%4) .. 4*(c%4)+4.
Each core computes q/k/v projections for its own head slice, the 4 attention
maps, and a partial output (its heads' contribution through Wo). The host
sums the 4 partials per batch.

Pipeline structure (per core): 8 softmax "ladders" ordered hp-major
(L = hp*4 + qc over head-pairs hp and 512-query chunks qc). The pre-phase
computes only the k-projection for head-pair 0 (ot=0) plus the first query
chunk, so ladder 0 starts ~10us in; all remaining q/k projections and the
Wo output projections are spread as PE inserts across later ladders where
the exp-activation (the ACT-bound inner loop) leaves tensor-engine slack.

Scores are computed transposed (S^T[k, q]) in f16 so softmax-exp output
feeds the AV matmul directly; softmax denominators ride along as a 65th
"ones" row of the V stationary, are broadcast to 64 partitions with a
0-stride DMA, inverted on DVE, and applied as an elementwise multiply.
"""

import os
import ml_dtypes
import numpy as np

SL, BS, D = 2048, 2, 1024
H, DH = 16, 64
NCORES = 8
HPC = 4            # heads per core
OD = HPC * DH      # 256 projected dims per core
DC = D // 128      # 8 contraction chunks
QC = SL // 512     # 4 query chunks of 512
KT = SL // 128     # 16 key tiles of 128

_NC = None
LAST_RESULT = None


def _build_nc():
    import concourse.mybir as mybir
    import concourse.tile as tile
    from concourse import bacc

    f32 = mybir.dt.float32
    f32r = mybir.dt.float32r
    bf16 = mybir.dt.bfloat16
    f16 = mybir.dt.float16
    EXP = mybir.ActivationFunctionType.Exp

    nc = bacc.Bacc(None, target_bir_lowering=False, debug=True)

    # x inputs are pre-chunked on the host so every SBUF tile is one
    # contiguous DRAM block: xq as [qc, d, 128, 512], xk/xv as
    # [cc2, d, 128, 1024], flattened to 2D.
    xqT = nc.dram_tensor("xqT", [QC * DC * 128, 512], f16, kind="ExternalInput")
    xkT = nc.dram_tensor("xkT", [2 * DC * 128, 1024], f16, kind="ExternalInput")
    xvT = nc.dram_tensor("xvT", [2 * DC * 128, 1024], f16, kind="ExternalInput")
    wqT = nc.dram_tensor("wqT", [128, DC * OD], f16, kind="ExternalInput")
    wkT = nc.dram_tensor("wkT", [128, DC * OD], f16, kind="ExternalInput")
    wvT = nc.dram_tensor("wvT", [128, DC * OD], f16, kind="ExternalInput")
    woT = nc.dram_tensor("woT", [128, 2 * D], f16, kind="ExternalInput")
    onesd = nc.dram_tensor("onesd", [128, 260], f32r, kind="ExternalInput")
    onesvd = nc.dram_tensor("onesvd", [128, 260], bf16, kind="ExternalInput")
    yT = nc.dram_tensor("yT", [D, SL], f32, kind="ExternalOutput")

    with tile.TileContext(nc) as tc:
        with (
            tc.tile_pool(name="wsb", bufs=1) as wsb,
            tc.tile_pool(name="qk", bufs=1) as qk,
            tc.tile_pool(name="vsb", bufs=1) as vsb,
            tc.tile_pool(name="xkp", bufs=16) as xkp,
            tc.tile_pool(name="xqp", bufs=32) as xqp,
            tc.tile_pool(name="xvp", bufs=16) as xvp,
            tc.tile_pool(name="esb", bufs=4) as esb,
            tc.tile_pool(name="otsb", bufs=8) as otsb,
            tc.tile_pool(name="ysb", bufs=4) as ysb,
            tc.tile_pool(name="yap", bufs=1) as yap,
            tc.tile_pool(name="otmp", bufs=2) as otmp,
            tc.tile_pool(name="avsb", bufs=2) as avsb,
            tc.tile_pool(name="pp", bufs=1, space="PSUM") as pp,
            tc.tile_pool(name="wp", bufs=2, space="PSUM") as wp,
            tc.tile_pool(name="avop", bufs=1, space="PSUM") as avop,
            tc.tile_pool(name="yp", bufs=1, space="PSUM") as yp,
        ):
            # --- persistent SBUF tensors ---
            wq_sb = wsb.tile([128, DC * OD], f16, tag="wq")  # [p, dc*256+od]
            wk_sb = wsb.tile([128, DC * OD], f16, tag="wk")
            wv_sb = wsb.tile([128, DC * OD], f16, tag="wv")
            wo_sb = wsb.tile([128, 2 * D], f16, tag="wo")    # [p, hp*1024+o]
            ones_sb = wsb.tile([128, 260], f32r, tag="ones")
            kT_sb = [qk.tile([128, SL], f16, tag=f"kT{ot}", name=f"kT{ot}") for ot in range(2)]
            qT_sb = [qk.tile([128, SL], f16, tag=f"qT{ot}", name=f"qT{ot}") for ot in range(2)]
            v_sb = [vsb.tile([128, 260], bf16, tag=f"v{t}", name=f"v{t}") for t in range(KT)]

            def load_w(dst, src):
                nc.sync.dma_start(out=dst[:], in_=src[:])

            def load_x(pool, tg, xdram, cc, dt_=f16):
                tiles = []
                for d in range(DC):
                    r = (cc * DC + d) * 128
                    t = pool.tile([128, 512], dt_, tag=tg, name="x")
                    nc.sync.dma_start(out=t[:], in_=xdram[r:r + 128, 0:512])
                    tiles.append(t[:])
                return tiles

            def load_x2(pool, tg, xdram, cc2, dt_=f16):
                # each [128, 1024] tile is one contiguous DRAM block
                ev, od = [], []
                for d in range(DC):
                    r = (cc2 * DC + d) * 128
                    t = pool.tile([128, 1024], dt_, tag=tg, name="x")
                    nc.sync.dma_start(out=t[:], in_=xdram[r:r + 128, 0:1024])
                    ev.append(t[:, 0:512])
                    od.append(t[:, 512:1024])
                return ev, od

            qp_ps = {}

            def proj_qk_quarter(w_sb, dst, cc, xtiles, ot, half):
                if half == 0:
                    qp_ps[(cc, ot)] = pp.tile([128, 512], f32, tag="pp",
                                              name="ps")
                ps = qp_ps[(cc, ot)]
                for d in range(half * 4, half * 4 + 4):
                    nc.tensor.matmul(
                        ps[:],
                        (w_sb[:, d * OD + ot * 128: d * OD + (ot + 1) * 128]),
                        (xtiles[d][:]),
                        start=(d == 0), stop=(d == DC - 1))
                if half == 1:
                    nc.vector.tensor_copy(dst[ot][:, cc * 512:(cc + 1) * 512],
                                          ps[:])

            def proj_tile(w_sb, dst, cc, xtiles, ot):
                proj_qk_quarter(w_sb, dst, cc, xtiles, ot, 0)
                proj_qk_quarter(w_sb, dst, cc, xtiles, ot, 1)

            def emit_wo_piece(qc_, ot_tiles, pool, ptag, o8):
                Y = pool.tile([128, 512], f32, tag=ptag, name="Y")
                for hp in range(2):
                    nc.tensor.matmul(
                        Y[:],
                        (wo_sb[:, hp * D + o8 * 128: hp * D + (o8 + 1) * 128]),
                        (ot_tiles[hp][:]),
                        start=(hp == 0), stop=(hp == 1))
                ys = ysb.tile([128, 512], f32, tag="ys", name="ys")
                nc.vector.tensor_copy(ys[:], Y[:])
                nc.sync.dma_start(
                    out=yT[o8 * 128:(o8 + 1) * 128, qc_ * 512:(qc_ + 1) * 512],
                    in_=ys[:])

            def emit_wo(qc_, ot_tiles, pools):
                for o8 in range(8):
                    pool, ptag = pools[o8 % len(pools)]
                    emit_wo_piece(qc_, ot_tiles, pool, ptag, o8)

            # --- pre-phase: warmup, k-proj ot=0, q-proj qc0 ot=0 ---
            nc.sync.dma_start(out=ones_sb[:], in_=onesd[:])
            load_w(wk_sb, wkT)
            warm = yp.tile([128, 512], f32, tag="yp", name="warm")
            for i in range(12):
                nc.tensor.matmul(warm[0:64, 0:256], ones_sb[:, 0:64],
                                 ones_sb[:, 0:256], start=(i == 0),
                                 stop=(i == 11))
            warms = ysb.tile([64, 256], f32, tag="ys", name="warms")
            nc.vector.tensor_copy(warms[:], warm[0:64, 0:256])

            ev0, od0 = load_x2(xkp, 'xk', xkT, 0, f16)
            load_w(wq_sb, wqT)
            xq_t = {0: load_x(xqp, 'xq', xqT, 0, f16)}
            load_w(wv_sb, wvT)
            ev1, od1 = load_x2(xkp, 'xk', xkT, 1, f16)
            xk_cc = [ev0, od0, ev1, od1]

            proj_tile(wk_sb, kT_sb, 0, xk_cc[0], 0)
            proj_tile(wk_sb, kT_sb, 1, xk_cc[1], 0)
            proj_tile(wq_sb, qT_sb, 0, xq_t[0], 0)

            nc.sync.dma_start(out=wo_sb[:], in_=woT[:])
            _xv01 = load_x2(xvp, 'xv', xvT, 0, f16)
            xv_chunks = {0: _xv01[0], 1: _xv01[1]}

            def emit_vtile(t_):
                cc_, tt = divmod(t_, 4)
                xtiles = xv_chunks[cc_]
                nc.sync.dma_start(out=v_sb[t_][:], in_=onesvd[:])
                ps = yp.tile([128, OD], f32, tag="yp", name="vps")
                for d in range(DC):
                    nc.tensor.matmul(
                        ps[:],
                        (xtiles[d][:, tt * 128:(tt + 1) * 128]),
                        (wv_sb[:, d * OD:(d + 1) * OD]),
                        start=(d == 0), stop=(d == DC - 1))
                for h in range(4):
                    nc.vector.tensor_copy(
                        v_sb[t_][:, h * 65:h * 65 + 64],
                        ps[:, h * 64:(h + 1) * 64])

            # --- insert schedule -------------------------------------------
            inserts = {}

            def at(L_, kt_, fn):
                inserts.setdefault((L_, kt_), []).append(fn)

            def load_xq(qc_):
                xq_t[qc_] = load_x(xqp, 'xq', xqT, qc_, f16)

            # remaining k-proj chunks for ot=0 (needed by ladder-0 kt>=8)
            at(0, 0, (lambda: proj_tile(wk_sb, kT_sb, 2, xk_cc[2], 0)))
            at(0, 1, (lambda: load_xq(1)))
            at(0, 2, (lambda: proj_tile(wk_sb, kT_sb, 3, xk_cc[3], 0)))
            at(0, 6, (lambda: proj_tile(wq_sb, qT_sb, 1, xq_t[1], 0)))
            # k-proj ot=1 (needed by ladder 4) and remaining q-projections
            at(1, 0, (lambda: proj_tile(wk_sb, kT_sb, 0, xk_cc[0], 1)))
            at(1, 1, (lambda: load_xq(2)))
            at(1, 5, (lambda: proj_tile(wq_sb, qT_sb, 2, xq_t[2], 0)))
            at(2, 0, (lambda: proj_tile(wk_sb, kT_sb, 1, xk_cc[1], 1)))
            at(2, 1, (lambda: load_xq(3)))
            at(2, 4, (lambda: proj_tile(wq_sb, qT_sb, 3, xq_t[3], 0)))
            at(3, 0, (lambda: proj_tile(wk_sb, kT_sb, 2, xk_cc[2], 1)))
            at(3, 8, (lambda: proj_tile(wq_sb, qT_sb, 0, xq_t[0], 1)))
            at(4, 0, (lambda: proj_tile(wk_sb, kT_sb, 3, xk_cc[3], 1)))
            at(4, 8, (lambda: proj_tile(wq_sb, qT_sb, 1, xq_t[1], 1)))
            at(5, 0, (lambda: proj_tile(wq_sb, qT_sb, 2, xq_t[2], 1)))
            at(6, 0, (lambda: proj_tile(wq_sb, qT_sb, 3, xq_t[3], 1)))

            # Wo(qc3) hp0-halves are staged to SBUF during ladder 7 so the
            # tail after the last exp only runs the hp1 matmuls + adds.
            ya_sb = {}

            def emit_wo_qc3_hp0(o8):
                pool, ptag = ((yp, 'yp'), (pp, 'pp'))[o8 % 2]
                Y = pool.tile([128, 512], f32, tag=ptag, name="Ya")
                nc.tensor.matmul(
                    Y[:],
                    (wo_sb[:, o8 * 128:(o8 + 1) * 128]),
                    (OTs[(QC - 1, 0)][:]),
                    start=True, stop=True)
                ya = yap.tile([128, 512], f32, tag=f"ya{o8}", name="ya")
                nc.vector.tensor_copy(ya[:], Y[:])
                ya_sb[o8] = ya

            for _o in range(8):
                at(7, 8 + _o, (lambda o=_o: emit_wo_qc3_hp0(o)))

            OTs = {}

            def emit_scores(L_, kt_):
                hp_, qc_ = divmod(L_, QC)
                W = wp.tile([128, 1024], f32, tag="wp", name="W")
                for hip in range(2):
                    nc.tensor.matmul(
                        W[:, hip * 512:(hip + 1) * 512],
                        (kT_sb[hp_][hip * 64:(hip + 1) * 64,
                                     kt_ * 128:(kt_ + 1) * 128]),
                        (qT_sb[hp_][hip * 64:(hip + 1) * 64,
                                     qc_ * 512:(qc_ + 1) * 512]),
                        start=True, stop=True)
                return W

            Ws = {}

            # --- attention: 8 ladders, hp-major (L = hp*4 + qc) ---
            for L in range(2 * QC):
                hp, qc = divmod(L, QC)
                AVO = [avop.tile([65, 512], f32, tag=f"av{hip}", name="AVO")
                       for hip in range(2)]

                def emit_av(E_, kt_, AVO=AVO, hp=hp):
                    for hip in range(2):
                        nc.tensor.matmul(
                            AVO[hip][:],
                            (v_sb[kt_][:, (hp * 2 + hip) * 65:
                                         (hp * 2 + hip) * 65 + 65]),
                            (E_[:, hip * 512:(hip + 1) * 512]),
                            start=(kt_ == 0), stop=(kt_ == KT - 1))

                prev_E = None
                for kt in range(KT):
                    if (L, kt) == (0, 0):
                        Ws[(0, 0)] = emit_scores(0, 0)
                    W = Ws.pop((L, kt))
                    E = esb.tile([128, 1024], bf16, tag="E", name="E")
                    nc.scalar.activation(E[:], W[:], EXP)
                    # emit the NEXT scores pair right after the exp so it
                    # outranks AV/v-proj/insert matmuls in scheduler priority
                    # and the next exp is never starved behind them
                    if kt < KT - 1:
                        Ws[(L, kt + 1)] = emit_scores(L, kt + 1)
                    elif L < 2 * QC - 1:
                        Ws[(L + 1, 0)] = emit_scores(L + 1, 0)
                    if prev_E is not None:
                        emit_av(prev_E, kt - 1)
                    prev_E = E
                    if L == 0:
                        emit_vtile(kt)
                        if kt == 1:
                            _xv23 = load_x2(xvp, 'xv', xvT, 1, f16)
                            xv_chunks[2] = _xv23[0]
                            xv_chunks[3] = _xv23[1]
                    for fn in inserts.pop((L, kt), []):
                        fn()
                emit_av(prev_E, KT - 1)

                # evacuate accumulators promptly, then defer the normalize
                # chain into the next ladder
                avs_pair = []
                for hip in range(2):
                    avs = avsb.tile([65, 512], f32, tag="avs", name="avs")
                    nc.vector.tensor_copy(avs[:], AVO[hip][:])
                    avs_pair.append(avs)
                OT = otsb.tile([128, 512], f16, tag="ot", name="OT")
                OTs[(qc, hp)] = OT

                def chain(hip, avs_pair=avs_pair, OT=OT):
                    avs = avs_pair[hip]
                    srow = ysb.tile([1, 512], f32, tag="ys", name="srow")
                    nc.sync.dma_start(out=srow[:], in_=avs[64:65, :])
                    sumsb = ysb.tile([64, 512], f32, tag="ys", name="sumsb")
                    nc.gpsimd.partition_broadcast(sumsb[:], srow[:])
                    BCs = ysb.tile([64, 512], f32, tag="ys", name="BCs")
                    nc.vector.reciprocal_approx_fast(BCs[:], sumsb[:])
                    if hip == 0:
                        nc.vector.tensor_mul(OT[0:64, :], avs[0:64, :], BCs[:])
                    else:
                        OTt = otmp.tile([64, 512], f16, tag="otmp", name="OTt")
                        nc.vector.tensor_mul(OTt[:], avs[0:64, :], BCs[:])
                        nc.sync.dma_start(out=OT[64:128, :], in_=OTt[:])

                if L < 2 * QC - 1:
                    at(L + 1, 1, (lambda c=chain: c(0)))
                    at(L + 1, 3, (lambda c=chain: c(1)))
                else:
                    # hip1's chain has the extra OT-assembly DMA hop; start
                    # it first so the tail Wo isn't waiting on it
                    chain(1)
                    chain(0)

                # Wo(qc) can start once OT(qc, hp=1) is written: its chain
                # runs at ladder L'=4+qc+1 slots 1/3, so spread the pieces
                # over slots 5..12 of that ladder, alternating PSUM pools.
                if hp == 1 and qc < QC - 1:
                    for o8 in range(8):
                        pool, ptag = ((yp, 'yp'), (pp, 'pp'))[o8 % 2]
                        at(L + 1, 5 + o8,
                           (lambda q=qc, o=o8, p=pool, pt=ptag:
                            emit_wo_piece(q, [OTs[(q, 0)], OTs[(q, 1)]],
                                          p, pt, o)))

            # tail: hp1 halves of Wo(qc3), added to the staged hp0 partials
            tail_pools = [(yp, 'yp'), (pp, 'pp'), (avop, 'av0'), (avop, 'av1')]
            for o8 in range(8):
                pool, ptag = tail_pools[o8 % 4]
                Y = pool.tile([128, 512], f32, tag=ptag, name="Yb")
                nc.tensor.matmul(
                    Y[:],
                    (wo_sb[:, D + o8 * 128:D + (o8 + 1) * 128]),
                    (OTs[(QC - 1, 1)][:]),
                    start=True, stop=True)
                ys = ysb.tile([128, 512], f32, tag="ys", name="ys")
                nc.vector.tensor_add(ys[:], ya_sb[o8][:], Y[:])
                nc.sync.dma_start(
                    out=yT[o8 * 128:(o8 + 1) * 128,
                           (QC - 1) * 512:QC * 512],
                    in_=ys[:])

    nc.compile()
    return nc


def _get_nc():
    global _NC
    if _NC is None:
        _NC = _build_nc()
    return _NC


def _host_fallback(query, keys, values, mask, Wq, Wk, Wv, Wo):
    # Exact reference math in numpy; only used if mask has zeros (off-spec).
    q = (query @ Wq.T).reshape(SL, BS, H, DH)
    k = (keys @ Wk.T).reshape(SL, BS, H, DH)
    v = (values @ Wv.T).reshape(SL, BS, H, DH)
    out = np.zeros((SL, BS, H * DH), np.float32)
    for b in range(BS):
        for h in range(H):
            s = q[:, b, h, :] @ k[:, b, h, :].T
            s = np.where(mask[0, 0] == 0, np.float32(-1e20), s)
            s = s - s.max(axis=-1, keepdims=True)
            p = np.exp(s)
            p /= p.sum(axis=-1, keepdims=True)
            out[:, b, h * DH:(h + 1) * DH] = p @ v[:, b, h, :]
    return out @ Wo.T


def _enable_trace_support():
    """Install the antenv.axon_hooks shim so trace=True works under axon."""
    import sys
    import types
    import antenv
    if "antenv.axon_hooks" in sys.modules:
        return
    hookmod = types.ModuleType("antenv.axon_hooks")
    _hook = [None]
    hookmod.set_axon_ntff_profile_hook = lambda h: _hook.__setitem__(0, h)
    hookmod.get_axon_ntff_profile_hook = lambda: _hook[0]
    antenv.axon_hooks = hookmod
    sys.modules["antenv.axon_hooks"] = hookmod
    try:
        from trn_agent_boot.trn_boot import _ntff_profile_via_ctypes
        hookmod.set_axon_ntff_profile_hook(
            _ntff_profile_via_ctypes("/opt/axon/libaxon_pjrt.so"))
    except Exception:
        pass
    import concourse.bass_utils as bu
    bu.upload_artifacts = lambda tmpdir: tmpdir


def _w_sb_layout(Wslice):
    # [256 od, 1024 D] -> [128 p, dc*256+od]
    return np.ascontiguousarray(
        Wslice.reshape(OD, DC, 128).transpose(2, 1, 0).reshape(128, DC * OD))


def _wo_sb_layout(WoSlice):
    # [1024 o, 256 hd] -> [128 p, hp*1024+o]
    return np.ascontiguousarray(
        WoSlice.reshape(D, 2, 128).transpose(2, 1, 0).reshape(128, 2 * D))


def kernel(query, keys, values, mask, Wq, Wk, Wv, Wo):
    query = np.asarray(query, np.float32)
    keys = np.asarray(keys, np.float32)
    values = np.asarray(values, np.float32)
    mask = np.asarray(mask)
    Wq = np.asarray(Wq, np.float32)
    Wk = np.asarray(Wk, np.float32)
    Wv = np.asarray(Wv, np.float32)
    Wo = np.asarray(Wo, np.float32)

    if (mask == 0).any():
        return _host_fallback(query, keys, values, mask, Wq, Wk, Wv, Wo)

    trace = bool(int(os.environ.get("KERNEL_TRACE", "0")))
    if trace:
        _enable_trace_support()

    from concourse.bass_utils import run_bass_kernel_spmd

    nc = _get_nc()
    in_maps = []
    for c in range(NCORES):
        b, hg = divmod(c, 4)
        hs = hg * OD
        xq_c = query[:, b, :].T.reshape(DC, 128, QC, 512).transpose(
            2, 0, 1, 3).reshape(QC * DC * 128, 512)
        xk_c = keys[:, b, :].T.reshape(DC, 128, 2, 1024).transpose(
            2, 0, 1, 3).reshape(2 * DC * 128, 1024)
        xv_c = values[:, b, :].T.reshape(DC, 128, 2, 1024).transpose(
            2, 0, 1, 3).reshape(2 * DC * 128, 1024)
        in_maps.append({
            "xqT": np.ascontiguousarray(xq_c).astype(np.float16),
            "xkT": np.ascontiguousarray(xk_c).astype(np.float16),
            "xvT": np.ascontiguousarray(xv_c).astype(np.float16),
            "wqT": _w_sb_layout(Wq[hs:hs + OD, :]).astype(np.float16),
            "wkT": _w_sb_layout(Wk[hs:hs + OD, :]).astype(np.float16),
            "wvT": _w_sb_layout(Wv[hs:hs + OD, :]).astype(np.float16),
            "woT": _wo_sb_layout(Wo[:, hs:hs + OD]).astype(np.float16),
            "onesd": np.ones((128, 260), np.float32),
            "onesvd": np.ones((128, 260), ml_dtypes.bfloat16),
        })

    res = run_bass_kernel_spmd(nc, in_maps, core_ids=list(range(NCORES)),
                               trace=trace)
    global LAST_RESULT
    LAST_RESULT = res

    out = np.zeros((SL, BS, D), np.float32)
    for c in range(NCORES):
        b = c // 4
        out[:, b, :] += res.results[c]["yT"].T
    return out
